# revision 28
# baseline (speedup 1.0000x reference)
"""Trainium2 Bass kernel for a single causal attention head.

Problem: x:(8,2048,1024) f32, per-head projections wq/wk/wv:(64,1024),
biases (64,). Output: softmax(causal(q k^T / sqrt(64))) @ v : (8,2048,64).

Strategy:
  - Data-parallel: batch b -> core b (8 cores, 1 batch each).
  - Host prep: x[b] transposed to xT:(1024,2048) fp16 (contraction dim D on
    SBUF partitions); Q/K weights shipped stacked as [wq|wk] (fp16, 1/sqrt(64)
    folded into wq).
  - Device (per core):
      * qk1 = [wq|wk]^T.T @ xT: rows 0-63 = Q^T, rows 64-127 = K^T (PSUM
        accumulate over 8 d-tiles, fp16 matmuls, N=512 chunks).
      * qk2 = half-swapped copy of qk1 (SBUF->SBUF DMA): K^T on rows 0-63,
        Q^T on rows 64-127. Both copies exist on both partition halves ->
        scores for TWO k-tiles run concurrently via PE row packing.
      * vT transposed back to (T,64) tiles via PE transpose, augmented with a
        ones column (softmax denominator rides along the PV matmul).
      * S^T[j,i] = sum_h K^T[h,j] Q^T[h,i] transposed-scores layout; P^T =
        exp(S^T) on ACT, one [128,1024] instr per k-tile pair; causal mask =
        GPSIMD affine_select zeroing P^T above the diagonal (identical
        result: zeros add nothing to numerator or denominator).
      * O^T_aug[65, T] accumulated in PSUM over k-tiles; row 64 = sum_j P^T.
      * causal skip: k-tiles entirely above the diagonal never computed.
      * attention for chunk ci is emitted right after projection chunk ci, so
        exp/PV overlap later projections instead of serializing at the end.
  - Host post: out[b] = (O^T[0:64] / O^T[64:65]).T  (softmax normalization).
"""

import numpy as np

B, T, D, HD = 8, 2048, 1024, 64
P = 128          # SBUF partitions
CH = 512         # q-chunk (matmul moving dim)
NCH = T // CH    # 4
DT = D // P      # 8 d-tiles
NKT = T // P     # 16 k-tiles
HT = T // 2      # xT half-tile width

LAST_RESULTS = None      # BassKernelResults of the most recent run (for test.py)


def _build_module(legalize=True):
    import concourse.bass as bass
    import concourse.mybir as mybir
    from concourse.tile import TileContext

    from concourse.masks import make_identity
    F32 = mybir.dt.float32
    F16 = mybir.dt.float16

    nc = bass.Bass("TRN2", target_bir_lowering=True)

    xT = nc.dram_tensor("xT", (D, T), F16, kind="ExternalInput")
    w1 = nc.dram_tensor("w1", (D, P), F16, kind="ExternalInput")   # [wq*s | wk]^T
    wv = nc.dram_tensor("wv", (D, HD), F16, kind="ExternalInput")  # wv^T
    b1 = nc.dram_tensor("b1", (P, 1), F32, kind="ExternalInput")   # [bq*s; bk]
    bv = nc.dram_tensor("bv", (P, 1), F32, kind="ExternalInput")  # [bv; bv]
    outT = nc.dram_tensor("outT", (HD + 1, T), F32, kind="ExternalOutput")

    with TileContext(nc) as tc:
        with (
            tc.tile_pool(name="const", bufs=1) as const,
            tc.tile_pool(name="acts", bufs=1) as acts,
            tc.tile_pool(name="proj_ps", bufs=2, space="PSUM") as proj_ps,
            tc.tile_pool(name="tr_ps", bufs=1, space="PSUM") as tr_ps,
            tc.tile_pool(name="s_ps", bufs=2, space="PSUM") as s_ps,
            tc.tile_pool(name="o_ps", bufs=1, space="PSUM") as o_ps,
            tc.tile_pool(name="pwork", bufs=6) as pwork,
            tc.tile_pool(name="owork", bufs=2) as owork,
        ):
            # ---- weights / biases first (small), then x half-tiles in
            # consumption order so chunk-0 projections start early ----
            ident = const.tile([P, P], F32, name="ident")
            make_identity(nc, ident)

            w1_sb = const.tile([P, DT, P], F16, name="w1_sb")
            nc.sync.dma_start(out=w1_sb[:], in_=w1.rearrange("(n p) h -> p n h", p=P))

            # xT as 4 quarter-T loads (1MB each == one q-chunk's needs), in
            # consumption order; chunk-0 projections gate on just w1 + xq0.
            # b1 lands before xq1 so the chunk-0 bias-add isn't held back.
            xr = xT.rearrange("(n p) t -> p n t", p=P)
            xq = []
            for ci in range(NCH):
                t = const.tile([P, DT, CH], F16, name=f"xq{ci}")
                nc.sync.dma_start(out=t[:], in_=xr[:, :, ci * CH:(ci + 1) * CH])
                xq.append(t)
                if ci == 0:
                    b1_sb = const.tile([P, 1], F32, name="b1_sb")
                    nc.sync.dma_start(out=b1_sb[:], in_=b1[:, :])
                    wv_sb = const.tile([P, DT, HD], F16, name="wv_sb")
                    nc.sync.dma_start(
                        out=wv_sb[:], in_=wv.rearrange("(n p) h -> p n h", p=P))
                    bv_sb = const.tile([P, 1], F32, name="bv_sb")
                    nc.sync.dma_start(out=bv_sb[:], in_=bv[:, :])


            # HAM warm-up: throwaway matmuls on a memset scratch tile (ready
            # ~4us before any DMA lands) keep the PE busy through its 3.4us
            # activity window, so every real matmul runs at the full 2.4 GHz
            # clock. 18 of them end ~2us before the first real group starts.
            wscr = const.tile([P, CH], F16, name="wscr")
            nc.vector.memset(wscr[:], 0.0)
            for wu in range(18):
                pswu = proj_ps.tile([P, CH], F32, name="warm", tag="proj")
                nc.tensor.matmul(pswu[:], wscr[:, 0:P], wscr[:],
                                 start=True, stop=True)

            # ---- activations ----
            # qk1: rows 0-63 = Q^T, rows 64-127 = K^T; qk2: swapped halves.
            qk1 = acts.tile([P, T], F16, name="qk1")
            qk2 = acts.tile([P, T], F16, name="qk2")
            vT = acts.tile([HD, T], F32, name="vT")
            v_aug = acts.tile([P, NKT, HD + 1], F16, name="v_aug")
            nc.vector.memset(v_aug[:, :, HD], 1.0)

            def qk_chunk(ci):
                cs = slice(ci * CH, (ci + 1) * CH)
                rhs = xq[ci]
                ps = proj_ps.tile([P, CH], F32, name="proj", tag="proj")
                for d in range(DT):
                    nc.tensor.matmul(ps[:], w1_sb[:, d, :], rhs[:, d, :],
                                     start=(d == 0), stop=(d == DT - 1))
                nc.vector.tensor_scalar_add(qk1[:, cs], ps[:], b1_sb[:])
                # half-swapped copy: qk2 = [K^T; Q^T]. 64-partition DVE ops
                # read any aligned src half and write either dest half.
                nc.vector.tensor_copy(qk2[0:HD, cs], qk1[HD:P, cs])
                nc.vector.tensor_copy(qk2[HD:P, cs], qk1[0:HD, cs])

            def v_pair(ca, cb):
                # V projections for two chunks col-packed: chunk ca on array
                # columns 0-63, chunk cb on columns 64-127 -> the matmul pairs
                # overlap in the PE array; outputs land in disjoint halves of
                # one PSUM bank.
                psv = proj_ps.tile([P, CH], F32, name="projv", tag="proj")
                for d in range(DT):
                    nc.tensor.matmul(psv[0:HD, :], wv_sb[:, d, :], xq[ca][:, d, :],
                                     start=(d == 0), stop=(d == DT - 1))
                    nc.tensor.matmul(psv[HD:P, :], wv_sb[:, d, :], xq[cb][:, d, :],
                                     start=(d == 0), stop=(d == DT - 1))
                nc.vector.tensor_scalar_add(
                    vT[:, ca * CH:(ca + 1) * CH], psv[0:HD, :], bv_sb[0:HD])
                nc.vector.tensor_scalar_add(
                    vT[:, cb * CH:(cb + 1) * CH], psv[HD:P, :], bv_sb[HD:P])
                for tt in range(4 * ca, 4 * ca + 8):
                    tp = tr_ps.tile([P, HD], F32, name="vtr", tag="vtr")
                    nc.tensor.transpose(tp[:], vT[:, tt * P:(tt + 1) * P],
                                        ident[:HD, :HD])
                    nc.vector.tensor_copy(v_aug[:, tt, 0:HD], tp[:])

            def attn_chunk(ci):
                cs = slice(ci * CH, (ci + 1) * CH)
                nkt = 4 * (ci + 1)
                ops = o_ps.tile([HD + 1, CH], F32, name="oacc", tag="oacc")
                for j in range(nkt // 2):
                    ka, kb = 2 * j, 2 * j + 1
                    s2 = s_ps.tile([P, 2 * CH], F32, name="sT", tag="sT")
                    # rows 0-63 of the array: K^T from qk2, Q^T from qk1
                    nc.tensor.matmul(s2[:, 0:CH], qk2[0:HD, ka * P:(ka + 1) * P],
                                     qk1[0:HD, cs], start=True, stop=True)
                    # rows 64-127: K^T from qk1, Q^T from qk2 (concurrent)
                    nc.tensor.matmul(s2[:, CH:2 * CH], qk1[HD:P, kb * P:(kb + 1) * P],
                                     qk2[HD:P, cs], start=True, stop=True)
                    pt = pwork.tile([P, 2 * CH], F16, name="pT", tag="pT")
                    nc.scalar.activation(pt[:], s2[:],
                                         mybir.ActivationFunctionType.Exp)
                    # causal mask: zero P^T where key > query (diagonal band)
                    for half, kt in ((0, ka), (1, kb)):
                        delta = kt * P - ci * CH
                        if delta >= 0:
                            nc.gpsimd.affine_select(
                                out=pt[:, half * CH:(half + 1) * CH],
                                in_=pt[:, half * CH:(half + 1) * CH],
                                compare_op=mybir.AluOpType.is_ge, fill=0.0,
                                base=-delta, pattern=[[1, CH]],
                                channel_multiplier=-1,
                            )
                    nc.tensor.matmul(ops[:], v_aug[:, ka, :], pt[:, 0:CH],
                                     start=(j == 0), stop=False)
                    nc.tensor.matmul(ops[:], v_aug[:, kb, :], pt[:, CH:2 * CH],
                                     start=False, stop=(j == nkt // 2 - 1))
                osb = owork.tile([HD + 1, CH], F32, name="osb", tag="osb")
                nc.vector.tensor_copy(osb[:], ops[:])
                nc.sync.dma_start(out=outT[:, cs], in_=osb[:])

            qk_chunk(0)
            v_pair(0, 1)
            attn_chunk(0)
            qk_chunk(1)
            attn_chunk(1)
            qk_chunk(2)
            v_pair(2, 3)
            attn_chunk(2)
            qk_chunk(3)
            attn_chunk(3)

    if legalize:
        _legalize_waits(nc, mybir)
    return nc


def _legalize_waits(nc, mybir):
    """Split multi-wait instructions for the XLA-route walrus codegen.

    The TPB EVENTS struct holds one semaphore wait per instruction and this
    pipeline's codegen refuses >1. Hoist extra waits onto standalone
    EventSemaphore instructions on the same engine queue right before the
    instruction - semantically identical, the queue stalls there.
    """
    n = 0
    for f in nc.m.functions:
        for b in f.blocks:
            out = []
            changed = False
            for inst in b.instructions:
                si = inst.sync_info
                waits = list(si.on_wait) if si is not None and si.on_wait else []
                if len(waits) > 1:
                    changed = True
                    for w in waits[:-1]:
                        n += 1
                        out.append(mybir.InstEventSemaphore(
                            name=f"waitfix{n}_{inst.name}",
                            engine=inst.engine,
                            sync_info=mybir.SyncInfo(on_wait=[w], on_update=[]),
                        ))
                    inst.sync_info = mybir.SyncInfo(
                        on_wait=waits[-1:],
                        on_update=list(si.on_update or []),
                    )
                out.append(inst)
            if changed:
                b.instructions = out
    return n


def kernel(x, wq, bq, wk, bk, wv, bv):
    global LAST_RESULTS
    import os
    os.environ.setdefault("JAX_PLATFORMS", "")
    from concourse.bass_utils import run_bass_kernel_spmd

    x = np.asarray(x, dtype=np.float32)
    s = np.float32(1.0 / np.sqrt(HD))
    wq_s = np.asarray(wq, np.float32) * s
    wk_f = np.asarray(wk, np.float32)
    w1 = np.ascontiguousarray(np.concatenate([wq_s, wk_f], 0).T.astype(np.float16))
    wv_c = np.ascontiguousarray(np.asarray(wv, np.float32).T.astype(np.float16))
    b1 = np.ascontiguousarray(
        np.concatenate([np.asarray(bq, np.float32) * s,
                        np.asarray(bk, np.float32)]).reshape(P, 1))
    bv_f = np.asarray(bv, np.float32)
    bv_c = np.ascontiguousarray(np.concatenate([bv_f, bv_f]).reshape(P, 1))
    xT = np.ascontiguousarray(np.swapaxes(x, 1, 2).astype(np.float16))  # (B, D, T)

    nc = _build_module()
    in_maps = [
        {"xT": xT[b], "w1": w1, "wv": wv_c, "b1": b1, "bv": bv_c}
        for b in range(B)
    ]
    res = None
    for attempt in range(3):
        try:
            res = run_bass_kernel_spmd(nc, in_maps, core_ids=list(range(B)))
            break
        except Exception:
            # transient device wedges (NRT_EXEC_UNIT_UNRECOVERABLE) happen;
            # rebuild the module and retry on a clean execution
            if attempt == 2:
                raise
            nc = _build_module()
    LAST_RESULTS = res

    out = np.empty((B, T, HD), dtype=np.float32)
    for b in range(B):
        oT = res.results[b]["outT"]  # (65, T): rows 0..63 = O^T, row 64 = denom
        out[b] = (oT[:HD] / oT[HD:HD + 1]).T
    return out
